# revision 1
# baseline (speedup 1.0000x reference)
import numpy as np
import concourse.bass as bass
import concourse.bacc as bacc_mod
import concourse.mybir as mybir
from concourse import tile
from concourse.bass_utils import run_bass_kernel_spmd

B, I, K, O, D = 128, 1152, 8, 32, 16
NC = 8
IL = I // NC          # 144 capsules per core
OD = O * D            # 512
CH = 4                # i-chunk size
NCH = IL // CH        # 24 chunks
EPS = 1e-8
NROUTES = 3

F32 = mybir.dt.float32
BF16 = mybir.dt.bfloat16
ADD = mybir.AluOpType.add
MULT = mybir.AluOpType.mult
AF = mybir.ActivationFunctionType
AX = mybir.AxisListType


def _build():
    nc = bacc_mod.Bacc()
    wc_d = nc.declare_dram_parameter("wc", [K, IL, B + OD], BF16,
                                     isOutput=False)
    id_d = nc.declare_dram_parameter("ident", [B, B], BF16, isOutput=False)
    v_d = nc.declare_dram_parameter("vout", [B, OD], F32, isOutput=True)
    # collective bounce buffers (unique per route: avoids DMA reuse waits)
    ar_in = [nc.dram_tensor(f"ar_in{r}", [B, OD], F32) for r in range(3)]
    ar_out = [nc.dram_tensor(f"ar_out{r}", [B, OD], F32) for r in range(3)]

    with tile.TileContext(nc) as tc:
        with (
            tc.tile_pool(name="big", bufs=1) as big,
            tc.tile_pool(name="ld", bufs=2) as ld,
            tc.tile_pool(name="work", bufs=2) as work,
            tc.tile_pool(name="small", bufs=1) as small,
            tc.tile_pool(name="ps_a", bufs=4, space="PSUM") as ps_a,
            tc.tile_pool(name="ps_z", bufs=2, space="PSUM") as ps_z,
            tc.tile_pool(name="ps_s", bufs=1, space="PSUM") as ps_s,
        ):
            # persistent tiles
            xh = big.tile([B, IL * OD], BF16, tag="xh")      # 147KB/part
            ident = small.tile([B, B], BF16, tag="id")
            nc.sync.dma_start(out=ident[:], in_=id_d[:])
            zc = small.tile([B, 1], F32, tag="zc")
            nc.vector.memset(zc[:], 0.0)
            nc.const_aps.aps[(F32, 0.0)] = zc[:]
            zbuf = big.tile([B, IL * O], F32, tag="z")        # 18KB/part (z then e)
            cbuf = big.tile([B, IL * O], BF16, tag="c")       # 9KB/part
            u16 = small.tile([B, OD], BF16, tag="u16")
            vsum = small.tile([B, OD], F32, tag="vsum")       # v1+v2 accumulator
            sar = small.tile([B, OD], F32, tag="sar")         # allreduced s

            # ---------- phase A: x_hat + route-1 s (uniform c) ----------
            s_ps = ps_s.tile([B, OD], F32, tag="sps")
            BOD = B + OD
            for ic in range(NCH):
                w_t = ld.tile([K, CH * BOD], BF16, tag="wt")
                nc.gpsimd.dma_start(
                    out=w_t[:], in_=wc_d[:, ic * CH:(ic + 1) * CH, :])
                for j in range(CH):
                    i_g = ic * CH + j
                    xh_ps = ps_a.tile([B, OD], F32, tag="xhps")
                    nc.tensor.matmul(
                        xh_ps[:], w_t[:, j * BOD:j * BOD + B],
                        w_t[:, j * BOD + B:(j + 1) * BOD],
                        start=True, stop=True)
                    # evacuate to bf16 slab, alternate DVE/ACT
                    dst = xh[:, i_g * OD:(i_g + 1) * OD]
                    # DVE:ACT ~ 3:2 split matches their PSUM-copy rates
                    if i_g % 5 < 3:
                        nc.vector.tensor_copy(dst, xh_ps[:])
                    else:
                        nc.scalar.copy(dst, xh_ps[:])
                    # route-1 s accumulation: s1 = sum_i x_hat_i (uniform c)
                    nc.tensor.matmul(
                        s_ps[:], ident[:], dst,
                        start=(i_g == 0), stop=(i_g == IL - 1))

            def all_reduce_s(s_psum, scale, rno):
                s_loc = work.tile([B, OD], F32, tag="sq_t2")
                nc.scalar.mul(s_loc[:], s_psum[:], scale)
                nc.sync.dma_start(out=ar_in[rno][:], in_=s_loc[:])
                nc.gpsimd.collective_compute(
                    "AllReduce", ADD,
                    replica_groups=[list(range(NC))],
                    ins=[ar_in[rno][:]], outs=[ar_out[rno][:]])
                sar = small.tile([B, OD], F32, tag="sarX")
                nc.sync.dma_start(out=sar[:], in_=ar_out[rno][:])
                return sar

            def squash_to(vdst32, sar, roundno):
                # sar holds s [B, (o,d)]; compute v = s * q/((1+q)sqrt(q+eps))
                s3 = sar[:].rearrange("p (o d) -> p o d", o=O)
                t = work.tile([B, OD], F32, tag="sq_t2")
                nc.vector.tensor_mul(t[:], sar[:], sar[:])
                q = small.tile([B, O], F32, tag="qsq")
                nc.vector.tensor_reduce(
                    q[:], t[:].rearrange("p (o d) -> p o d", o=O),
                    axis=AX.X, op=ADD)
                qe = small.tile([B, O], F32, tag="qesq")
                nc.vector.tensor_scalar_add(qe[:], q[:], EPS)
                r = small.tile([B, O], F32, tag="rsq")
                nc.scalar.activation(r[:], qe[:], AF.Sqrt)
                t1 = small.tile([B, O], F32, tag="t1sq")
                nc.vector.scalar_tensor_tensor(
                    t1[:], q[:], 1.0, r[:], op0=ADD, op1=MULT)
                t2 = small.tile([B, O], F32, tag="t2sq")
                nc.vector.reciprocal(t2[:], t1[:])
                f = small.tile([B, O], F32, tag="fsq")
                nc.vector.tensor_mul(f[:], q[:], t2[:])
                fb = f[:].broadcast_to((B, O, D))
                nc.vector.tensor_mul(
                    vdst32[:].rearrange("p (o d) -> p o d", o=O), s3, fb)

            # ---------- route 1 ----------
            sar1 = all_reduce_s(s_ps, 1.0 / O, 0)
            squash_to(vsum, sar1, 1)
            nc.vector.tensor_copy(u16[:], vsum[:])   # u2 = v1 (bf16)

            # ---------- routes 2..3 ----------
            for rt in range(2, NROUTES + 1):
                last = rt == NROUTES
                # z = sum_d xhat * u  (DVE mult -> PE accum over d)
                ub = u16[:].rearrange("p (x o d) -> p x o d", x=1, o=O) \
                           .broadcast_to((B, CH, O, D))
                for ic in range(NCH):
                    y = work.tile([B, CH * OD], BF16, tag="y")
                    xs = xh[:, ic * CH * OD:(ic + 1) * CH * OD] \
                        .rearrange("p (i o d) -> p i o d", o=O, d=D)
                    nc.vector.tensor_mul(
                        y[:].rearrange("p (i o d) -> p i o d", o=O, d=D),
                        xs, ub)
                    z_ps = ps_z.tile([B, CH * O], F32, tag="zps")
                    yv = y[:].rearrange("p (i o d) -> p i o d", o=O, d=D)
                    for d in range(D):
                        nc.tensor.matmul(
                            z_ps[:], ident[:], yv[:, :, :, d],
                            start=(d == 0), stop=(d == D - 1))
                    # exp straight out of PSUM -> e (fp32, zbuf slab)
                    nc.scalar.activation(
                        zbuf[:, ic * CH * O:(ic + 1) * CH * O], z_ps[:],
                        AF.Exp)
                # softmax denom over o, then c = e * (1/sigma)
                sig = small.tile([B, IL], F32, tag="sig")
                nc.vector.tensor_reduce(
                    sig[:], zbuf[:].rearrange("p (i o) -> p i o", o=O),
                    axis=AX.X, op=ADD)
                rho = small.tile([B, IL], F32, tag="rho")
                nc.vector.reciprocal(rho[:], sig[:])
                rb = rho[:].rearrange("p (i x) -> p i x", x=1).broadcast_to((B, IL, O))
                nc.vector.tensor_mul(
                    cbuf[:].rearrange("p (i o) -> p i o", o=O),
                    zbuf[:].rearrange("p (i o) -> p i o", o=O), rb)
                # s = sum_i c * xhat  (DVE mult -> PE accum over i)
                s_ps2 = ps_s.tile([B, OD], F32, tag="sps")
                for ic in range(NCH):
                    sy = work.tile([B, CH * OD], BF16, tag="y")
                    cb = cbuf[:, ic * CH * O:(ic + 1) * CH * O] \
                        .rearrange("p (i o x) -> p i o x", o=O, x=1) \
                        .broadcast_to((B, CH, O, D))
                    xs = xh[:, ic * CH * OD:(ic + 1) * CH * OD] \
                        .rearrange("p (i o d) -> p i o d", o=O, d=D)
                    nc.vector.tensor_mul(
                        sy[:].rearrange("p (i o d) -> p i o d", o=O, d=D),
                        xs, cb)
                    for j in range(CH):
                        i_g = ic * CH + j
                        nc.tensor.matmul(
                            s_ps2[:], ident[:],
                            sy[:, j * OD:(j + 1) * OD],
                            start=(i_g == 0), stop=(i_g == IL - 1))
                sarR = all_reduce_s(s_ps2, 1.0, rt - 1)
                if last:
                    vout_t = small.tile([B, OD], F32, tag="vfin")
                    squash_to(vout_t, sarR, rt)
                    nc.sync.dma_start(out=v_d[:], in_=vout_t[:])
                else:
                    v2 = small.tile([B, OD], F32, tag="vfin")
                    squash_to(v2, sarR, rt)
                    nc.vector.tensor_add(vsum[:], vsum[:], v2[:])
                    nc.vector.tensor_copy(u16[:], vsum[:])  # u3 = v1+v2
    nc.compile()
    return nc


def _filter_bir(bir_json: bytes) -> bytes:
    """Drop same-ring WAW waits on DMAs (ring FIFO makes them redundant);
    the DIRECT2D descriptor only holds one wait command."""
    import json
    d = json.loads(bir_json)
    for fn in d.get("functions", []):
        for blk in fn.get("blocks", []):
            for inst in blk.get("instructions", []):
                if inst.get("opcode") != "DMACopy":
                    continue
                si = inst.get("sync_info") or {}
                waits = si.get("on_wait") or []
                if len(waits) <= 1:
                    continue
                ups = {u.get("ant_name") for u in (si.get("on_update") or [])}
                kept = [w for w in waits if w.get("ant_name") not in ups]
                if len(kept) < len(waits):
                    si["on_wait"] = kept
    return json.dumps(d).encode()


def _install_bir_filter():
    from concourse import bass2jax, bass_utils

    orig = bass_utils.compile_bir_kernel

    def patched(bir_json, tmpdir, neff_name="file.neff"):
        return orig(_filter_bir(bir_json), tmpdir, neff_name)

    bass2jax.compile_bir_kernel = patched


def _make_in_maps(x: np.ndarray, W: np.ndarray):
    ident = np.eye(B, dtype=np.float32)
    in_maps = []
    for c in range(NC):
        sl = slice(c * IL, (c + 1) * IL)
        xt = np.ascontiguousarray(
            x[:, sl, :].transpose(2, 1, 0)).astype(np.float32)  # [K, IL, B]
        wk = np.ascontiguousarray(
            W[sl].transpose(2, 0, 1, 3).reshape(K, IL, OD)).astype(np.float32)
        wc = np.concatenate([xt, wk], axis=2)  # [K, IL, B+OD]
        in_maps.append({"wc": _bf16(wc), "ident": _bf16(ident)})
    return in_maps


def kernel(x: np.ndarray, W: np.ndarray) -> np.ndarray:
    _install_bir_filter()
    nc = _build()
    in_maps = _make_in_maps(x, W)
    res = run_bass_kernel_spmd(nc, in_maps, list(range(NC)))
    v = np.asarray(res.results[0]["vout"], dtype=np.float32)
    return v.reshape(B, O, D)


def _bf16(a: np.ndarray):
    import jax.numpy as jnp
    return np.asarray(jnp.asarray(a, dtype=jnp.bfloat16))


if __name__ == "__main__":
    nc = _build()
    print("IR build OK")



# revision 2
# speedup vs baseline: 2958.7799x; 2958.7799x over previous
"""Capsule-layer dynamic routing on 8 TRN2 NeuronCores (Bass/Tile).

Sharding: input capsules I=1152 are split 144 per core; each core computes its
partial weighted sums s which are AllReduced (3x, once per routing iteration).
B=128 occupies the full SBUF partition dimension.

Kernel design (vs. the straightforward implementation):
- x_hat is materialized once in SBUF as bf16 with free-dim layout (i, d, o).
  With d-major layout both routing multiplies - y = x_hat * u (u broadcast
  over i) and sy = x_hat * c (c broadcast over d) - keep a packed innermost
  dimension, which qualifies for the DVE 2-byte 2x throughput mode.
- Route-1 s needs no x_hat: s1 = sum_ik x_bik W_ik(do) is computed by 9
  accumulating 128-deep matmuls on combined (i,k)-row staging tiles, so the
  first AllReduce + squash run concurrently with x_hat materialization.
  The uniform softmax weight 1/O is folded into the staged W copy.
- Phase A streams concat(x, W) in [K=8, i, B+(d,o)] chunks on the SP DMA
  queue; two x_hat matmuls share a 2-bank PSUM tile and are evacuated by
  Activation (2/3) and DVE (1/3).
- Per-chunk softmax: exp on Activation, denominator reduce + reciprocal on
  DVE, c-multiply on Pool (GPSIMD), pipelined inside the z-phase loop so the
  s-phase starts with no serial softmax gap.
- z = sum_d y via per-d accumulating identity matmuls (PE); s = sum_i sy via
  accumulating identity matmuls (PE). Long PE streams keep the tensor engine
  at its ramped p-state.
- AllReduce staging and the final output DMA are split across SP/Act queues.
Output v is produced in (d, o) layout; the host transposes to (o, d).
"""
import numpy as np
import concourse.bass as bass
import concourse.bacc as bacc_mod
import concourse.mybir as mybir
from concourse import tile
from concourse.bass_utils import run_bass_kernel_spmd

B, I, K, O, D = 128, 1152, 8, 32, 16
NC = 8
IL = I // NC          # 144 capsules per core
OD = O * D            # 512
NBLK = IL * K // 128  # 9 combined (i,k) row blocks
CH_A = 6              # i-chunk for phase A streaming
NCH_A = IL // CH_A    # 24
CH = 6                # i-chunk for routing
NCH = IL // CH        # 24
EPS = 1e-8
NROUTES = 3
BOD = B + OD

F32 = mybir.dt.float32
BF16 = mybir.dt.bfloat16
ADD = mybir.AluOpType.add
MULT = mybir.AluOpType.mult
AF = mybir.ActivationFunctionType
AX = mybir.AxisListType


def _build(collectives=True, reps=1):
    nc = bacc_mod.Bacc()
    x_d = nc.declare_dram_parameter("xc", [128, NBLK * B], BF16, isOutput=False)
    w_d = nc.declare_dram_parameter("wc", [128, NBLK * OD], BF16, isOutput=False)
    s_d = nc.declare_dram_parameter("sc", [K, IL * BOD], BF16, isOutput=False)
    id_d = nc.declare_dram_parameter("ident", [B, B], BF16, isOutput=False)
    v_d = nc.declare_dram_parameter("vout", [B, OD], F32, isOutput=True)
    ar_in = [nc.dram_tensor(f"ar_in{r}", [B, OD], F32) for r in range(3 * reps)]
    ar_out = [nc.dram_tensor(f"ar_out{r}", [B, OD], F32) for r in range(3 * reps)]

    with tile.TileContext(nc) as tc:
        with (
            tc.tile_pool(name="big", bufs=1) as big,
            tc.tile_pool(name="ld", bufs=2) as ld,
            tc.tile_pool(name="work", bufs=2) as work,
            tc.tile_pool(name="small", bufs=1) as small,
            tc.tile_pool(name="ps_a", bufs=2, space="PSUM") as ps_a,
            tc.tile_pool(name="ps_z", bufs=2, space="PSUM") as ps_z,
            tc.tile_pool(name="ps_s", bufs=1, space="PSUM") as ps_s,
        ):
            xh = big.tile([B, IL * OD], BF16, tag="xh")     # (i, d, o)
            ident = small.tile([B, B], BF16, tag="id")
            u16 = small.tile([B, OD], BF16, tag="u16")      # (d, o)
            vsum = small.tile([B, OD], F32, tag="vsum")

            nc.sync.dma_start(out=ident[:], in_=id_d[:])
            zc = small.tile([B, 1], F32, tag="zc")
            nc.vector.memset(zc[:], 0.0)
            nc.const_aps.aps[(F32, 0.0)] = zc[:]

            def all_reduce_s(s_psum, scale, rno):
                s_loc = work.tile([B, OD], F32, tag="arstage")
                nc.scalar.mul(s_loc[:], s_psum[:], scale)
                H = OD // 2
                nc.sync.dma_start(out=ar_in[rno][:, :H], in_=s_loc[:, :H])
                nc.scalar.dma_start(out=ar_in[rno][:, H:], in_=s_loc[:, H:])
                if collectives:
                    nc.gpsimd.collective_compute(
                        "AllReduce", ADD,
                        replica_groups=[list(range(NC))],
                        ins=[ar_in[rno][:]], outs=[ar_out[rno][:]])
                else:
                    nc.gpsimd.dma_start(out=ar_out[rno][:], in_=ar_in[rno][:])
                sar = small.tile([B, OD], F32, tag="sarX")
                nc.sync.dma_start(out=sar[:, :H], in_=ar_out[rno][:, :H])
                nc.scalar.dma_start(out=sar[:, H:], in_=ar_out[rno][:, H:])
                return sar

            def squash_to(vdst32, sar):
                # sar holds s [B, (d, o)]; v = s * q/((1+q)sqrt(q+eps))
                s3 = sar[:].rearrange("p (d o) -> p d o", d=D)
                t = work.tile([B, OD], F32, tag="sqt")
                nc.vector.tensor_mul(t[:], sar[:], sar[:])
                q = small.tile([B, O], F32, tag="qsq")
                nc.vector.tensor_reduce(
                    q[:], t[:].rearrange("p (d o) -> p o d", d=D),
                    axis=AX.X, op=ADD)
                qe = small.tile([B, O], F32, tag="qesq")
                nc.vector.tensor_scalar_add(qe[:], q[:], EPS)
                r = small.tile([B, O], F32, tag="rsq")
                nc.scalar.activation(r[:], qe[:], AF.Sqrt)
                t1 = small.tile([B, O], F32, tag="t1sq")
                nc.vector.scalar_tensor_tensor(
                    t1[:], q[:], 1.0, r[:], op0=ADD, op1=MULT)
                t2 = small.tile([B, O], F32, tag="t2sq")
                nc.vector.reciprocal(t2[:], t1[:])
                f = small.tile([B, O], F32, tag="fsq")
                nc.vector.tensor_mul(f[:], q[:], t2[:])
                fb = f[:].rearrange("p (x o) -> p x o", x=1).broadcast_to((B, D, O))
                nc.vector.tensor_mul(
                    vdst32[:].rearrange("p (d o) -> p d o", d=D), s3, fb)

            for _rep in range(reps):
                _aroff = 3 * _rep
                # combined (i,k)-row tiles for the early route-1 matmuls
                x_t = big.tile([128, NBLK * B], BF16, tag="xt")
                w_t = big.tile([128, NBLK * OD], BF16, tag="wz")
                nc.gpsimd.dma_start(out=x_t[:], in_=x_d[:])
                nc.gpsimd.dma_start(out=w_t[:], in_=w_d[:])

                # -------- route-1 s: 9 big matmuls (no x_hat needed) --------
                s_ps = ps_s.tile([B, OD], F32, tag="sps")
                for c in range(NBLK):
                    nc.tensor.matmul(
                        s_ps[:], x_t[:, c * B:(c + 1) * B],
                        w_t[:, c * OD:(c + 1) * OD],
                        start=(c == 0), stop=(c == NBLK - 1))

                # -------- phase A: x_hat materialization, (i, d, o) --------
                for ic in range(NCH_A):
                    wch = ld.tile([K, CH_A * BOD], BF16, tag="wch")
                    nc.sync.dma_start(
                        out=wch[:],
                        in_=s_d[:, ic * CH_A * BOD:(ic + 1) * CH_A * BOD])
                    for g in range(CH_A // 2):
                        xp = ps_a.tile([B, 2 * OD], F32, tag="xhps")
                        for j in range(2):
                            jj = 2 * g + j
                            nc.tensor.matmul(
                                xp[:, j * OD:(j + 1) * OD],
                                wch[:, jj * BOD:jj * BOD + B],
                                wch[:, jj * BOD + B:(jj + 1) * BOD],
                                start=True, stop=True)
                        i0 = ic * CH_A + 2 * g
                        dst = xh[:, i0 * OD:(i0 + 2) * OD]
                        if (ic * (CH_A // 2) + g) % 3 == 2:
                            nc.vector.tensor_copy(dst, xp[:])
                        else:
                            nc.scalar.copy(dst, xp[:])

                # -------- route 1 (overlaps phase A) --------
                sar1 = all_reduce_s(s_ps, 1.0, _aroff + 0)  # 1/O folded in wc
                squash_to(vsum, sar1)
                nc.vector.tensor_copy(u16[:], vsum[:])   # u2 = v1 (bf16)

                # -------- routes 2..3 --------
                zbuf = big.tile([B, IL * O], BF16, tag="wz")  # reuses w_t space
                for rt in range(2, NROUTES + 1):
                    last = rt == NROUTES
                    ub = u16[:].rearrange("p (x d o) -> p x d o", x=1, d=D) \
                               .broadcast_to((B, CH, D, O))
                    sig = small.tile([B, IL], F32, tag="sig")
                    rho = small.tile([B, IL], F32, tag="rho")
                    # z-phase with per-chunk softmax pipelined in
                    for ic in range(NCH):
                        y = work.tile([B, CH * OD], BF16, tag="y")
                        xs = xh[:, ic * CH * OD:(ic + 1) * CH * OD] \
                            .rearrange("p (i d o) -> p i d o", d=D, o=O)
                        nc.vector.tensor_mul(
                            y[:].rearrange("p (i d o) -> p i d o", d=D, o=O),
                            xs, ub)
                        z_ps = ps_z.tile([B, CH * O], F32, tag="zps")
                        yv = y[:].rearrange("p (i d o) -> p i d o", d=D, o=O)
                        for d in range(D):
                            nc.tensor.matmul(
                                z_ps[:], ident[:], yv[:, :, d, :],
                                start=(d == 0), stop=(d == D - 1))
                        zch = zbuf[:, ic * CH * O:(ic + 1) * CH * O]
                        nc.scalar.activation(zch, z_ps[:], AF.Exp)
                        sch = sig[:, ic * CH:(ic + 1) * CH]
                        nc.vector.tensor_reduce(
                            sch, zch.rearrange("p (i o) -> p i o", o=O),
                            axis=AX.X, op=ADD)
                        rch = rho[:, ic * CH:(ic + 1) * CH]
                        nc.vector.reciprocal(rch, sch)
                        rb = rch.rearrange("p (i x) -> p i x", x=1) \
                                .broadcast_to((B, CH, O))
                        zv = zch.rearrange("p (i o) -> p i o", o=O)
                        nc.gpsimd.tensor_mul(zv, zv, rb)   # c chunk in place
                    # s-phase
                    s_ps2 = ps_s.tile([B, OD], F32, tag="sps")
                    for ic in range(NCH):
                        sy = work.tile([B, CH * OD], BF16, tag="y")
                        cb = zbuf[:, ic * CH * O:(ic + 1) * CH * O] \
                            .rearrange("p (i x o) -> p i x o", x=1, o=O) \
                            .broadcast_to((B, CH, D, O))
                        xs = xh[:, ic * CH * OD:(ic + 1) * CH * OD] \
                            .rearrange("p (i d o) -> p i d o", d=D, o=O)
                        nc.vector.tensor_mul(
                            sy[:].rearrange("p (i d o) -> p i d o", d=D, o=O),
                            xs, cb)
                        for j in range(CH):
                            i_g = ic * CH + j
                            nc.tensor.matmul(
                                s_ps2[:], ident[:],
                                sy[:, j * OD:(j + 1) * OD],
                                start=(i_g == 0), stop=(i_g == IL - 1))
                    sarR = all_reduce_s(s_ps2, 1.0, _aroff + rt - 1)
                    if last:
                        vout_t = small.tile([B, OD], F32, tag="vfin")
                        squash_to(vout_t, sarR)
                        Q = OD // 4
                        nc.sync.dma_start(out=v_d[:, :Q], in_=vout_t[:, :Q])
                        nc.scalar.dma_start(
                            out=v_d[:, Q:2 * Q], in_=vout_t[:, Q:2 * Q])
                        nc.gpsimd.dma_start(
                            out=v_d[:, 2 * Q:3 * Q], in_=vout_t[:, 2 * Q:3 * Q])
                        nc.gpsimd.dma_start(
                            out=v_d[:, 3 * Q:], in_=vout_t[:, 3 * Q:])
                    else:
                        v2 = small.tile([B, OD], F32, tag="vfin")
                        squash_to(v2, sarR)
                        nc.vector.tensor_add(vsum[:], vsum[:], v2[:])
                        nc.vector.tensor_copy(u16[:], vsum[:])  # u3 = v1+v2
    nc.compile()
    return nc


def _filter_bir(bir_json: bytes) -> bytes:
    """Drop same-ring WAW waits on DMAs (ring FIFO makes them redundant);
    the DIRECT2D descriptor only holds one wait command."""
    import json
    d = json.loads(bir_json)
    for fn in d.get("functions", []):
        for blk in fn.get("blocks", []):
            for inst in blk.get("instructions", []):
                if inst.get("opcode") != "DMACopy":
                    continue
                si = inst.get("sync_info") or {}
                waits = si.get("on_wait") or []
                if len(waits) <= 1:
                    continue
                ups = {u.get("ant_name") for u in (si.get("on_update") or [])}
                kept = [w for w in waits if w.get("ant_name") not in ups]
                if len(kept) < len(waits):
                    si["on_wait"] = kept
    return json.dumps(d).encode()


def _install_bir_filter():
    from concourse import bass2jax, bass_utils

    orig = bass_utils.compile_bir_kernel

    def patched(bir_json, tmpdir, neff_name="file.neff"):
        return orig(_filter_bir(bir_json), tmpdir, neff_name)

    bass2jax.compile_bir_kernel = patched


def _bf16(a: np.ndarray):
    import jax.numpy as jnp
    return np.asarray(jnp.asarray(a, dtype=jnp.bfloat16))


def _make_in_maps(x: np.ndarray, W: np.ndarray):
    ident = np.eye(B, dtype=np.float32)
    in_maps = []
    for c in range(NC):
        sl = slice(c * IL, (c + 1) * IL)
        # combined rows r = i*8+k -> partition r%128, block r//128
        xt = x[:, sl, :].transpose(1, 2, 0).reshape(IL * K, B)
        wt = W[sl].transpose(0, 2, 3, 1).reshape(IL * K, OD) / O  # 1/O folded
        xt = np.ascontiguousarray(
            xt.reshape(NBLK, 128, B).transpose(1, 0, 2).reshape(128, NBLK * B))
        wt = np.ascontiguousarray(
            wt.reshape(NBLK, 128, OD).transpose(1, 0, 2).reshape(128, NBLK * OD))
        # streamed phase-A copy: [k, i, b | (d, o)]
        x8 = x[:, sl, :].transpose(2, 1, 0)                      # [K, IL, B]
        w8 = W[sl].transpose(2, 0, 3, 1).reshape(K, IL, OD)      # [K, IL, (d,o)]
        sc = np.concatenate([x8, w8], axis=2).reshape(K, IL * BOD)
        in_maps.append({"xc": _bf16(xt), "wc": _bf16(wt), "sc": _bf16(sc),
                        "ident": _bf16(ident)})
    return in_maps


def _unshard(res_list):
    v = np.asarray(res_list[0]["vout"], dtype=np.float32)
    return np.ascontiguousarray(v.reshape(B, D, O).transpose(0, 2, 1))


def kernel(x: np.ndarray, W: np.ndarray) -> np.ndarray:
    _install_bir_filter()
    nc = _build()
    in_maps = _make_in_maps(x, W)
    res = run_bass_kernel_spmd(nc, in_maps, list(range(NC)))
    return _unshard(res.results)


if __name__ == "__main__":
    nc = _build()
    print("IR build OK")
